# revision 12
# baseline (speedup 1.0000x reference)
"""ConnectivityLoss (MALIS) kernel for Trainium2.

Contract mirrors the reference nn.Module: the MALIS pair weights are
computed on host (union-find / Kruskal maximin over prediction-dependent
edge costs, exactly as the torch module does via skimage+malis on detached
CPU numpy arrays); the device computes the weighted squared loss
    loss = sum( y_pred^2 * w_neg + (20 - y_pred)^2 * w_pos )
as a Bass kernel, data-parallel across 8 NeuronCores (each core reduces
a 16384-element shard; host adds the 8x[128,1] partials).
"""

import numpy as np

import concourse.bass as bass
import concourse.mybir as mybir
from concourse.bass_utils import run_bass_kernel_spmd

WINDOW = 32
MALIS_LR = 1.0
MALIS_LR_POS = 1.0

N_CORES = 8
B, C, H, W = 2, 1, 256, 256
TOTAL = B * C * H * W          # 131072
PER_CORE = TOTAL // N_CORES    # 16384
P = 128                        # SBUF partitions
F = PER_CORE // P              # 128 free elements per partition


# ---------------------------------------------------------------------------
# Host-side MALIS weights (exact replica of the reference host computation)
# ---------------------------------------------------------------------------

def _make_node_indexes(w):
    idx = np.arange(w * w).reshape(w, w)
    h = np.stack([idx[:, :-1].ravel(), idx[:, 1:].ravel()])
    v = np.stack([idx[:-1, :].ravel(), idx[1:, :].ravel()])
    return np.concatenate([h, v], axis=1)  # [2, 2*w*(w-1)]


class _UF:
    def __init__(self, n):
        self.p = list(range(n))

    def find(self, x):
        p = self.p
        while p[x] != x:
            p[x] = p[p[x]]
            x = p[x]
        return x


try:
    from scipy import ndimage as _ndimage
    _LABEL8_STRUCT = np.ones((3, 3), dtype=bool)
except Exception:  # pragma: no cover
    _ndimage = None


def _label8(mask):
    """8-connectivity labeling: labels>=1 inside mask, 0 outside.

    Only the label *partition* (which pixels share a component) matters for
    the MALIS pair counts — label numbering never affects the result.
    """
    if _ndimage is not None:
        labels, _ = _ndimage.label(mask, structure=_LABEL8_STRUCT)
        return labels.astype(np.int64).ravel()
    h, w = mask.shape
    n = h * w
    uf = _UF(n)
    idx = np.arange(n).reshape(h, w)
    pairs = []
    for (a, b) in (
        (idx[:, :-1], idx[:, 1:]),
        (idx[:-1, :], idx[1:, :]),
        (idx[:-1, :-1], idx[1:, 1:]),
        (idx[:-1, 1:], idx[1:, :-1]),
    ):
        m = mask.ravel()[a.ravel()] & mask.ravel()[b.ravel()]
        pairs.append(np.stack([a.ravel()[m], b.ravel()[m]], 1))
    for a, b in np.concatenate(pairs, 0):
        ra, rb = uf.find(int(a)), uf.find(int(b))
        if ra != rb:
            uf.p[rb] = ra
    labels = np.zeros(n, dtype=np.int64)
    root_lab = {}
    nxt = 1
    flat = mask.ravel()
    for i in range(n):
        if flat[i]:
            r = uf.find(i)
            if r not in root_lab:
                root_lab[r] = nxt
                nxt += 1
            labels[i] = root_lab[r]
    return labels


def _malis_weights(seg, n1, n2, costs, pos):
    """MALIS pair counts per edge: Kruskal maximin, descending cost order.

    Exact-equivalent rewrite of the straightforward version: pair counts are
    symmetric in the two merged components (pos: sum_l |A_l|*|B_l|; neg:
    |A|*|B| - sum_l |A_l|*|B_l|), so merge direction and union-find internals
    are free — only the stable descending edge order matters.
    """
    n = seg.shape[0]
    parent = list(range(n))
    seg_l = seg.astype(np.int64).tolist()
    overlap = [({s: 1} if s else None) for s in seg_l]
    tot = [1 if s else 0 for s in seg_l]
    w = np.zeros(costs.shape[0], dtype=np.int64)
    order = np.argsort(-costs, kind="stable").tolist()
    n1l = n1.tolist()
    n2l = n2.tolist()
    for e in order:
        a = n1l[e]
        b = n2l[e]
        while parent[a] != a:
            parent[a] = parent[parent[a]]
            a = parent[a]
        while parent[b] != b:
            parent[b] = parent[parent[b]]
            b = parent[b]
        if a == b:
            continue
        oa = overlap[a]
        ob = overlap[b]
        if oa is None:
            parent[a] = b
            continue
        if ob is None:
            parent[b] = a
            continue
        ta = tot[a]
        tb = tot[b]
        if len(oa) >= len(ob):
            big, small = oa, ob
        else:
            big, small = ob, oa
        inner = 0
        for l, c in small.items():
            g = big.get(l)
            if g:
                inner += c * g
        w[e] = inner if pos else ta * tb - inner
        for l, c in small.items():
            big[l] = big.get(l, 0) + c
        if big is oa:
            parent[b] = a
            tot[a] = ta + tb
            overlap[b] = None
        else:
            parent[a] = b
            tot[b] = ta + tb
            overlap[a] = None
    return w


def _connectivity_weights(y_true_np, y_pred_np, window):
    B_, C_, H_, W_ = y_pred_np.shape
    ni = _make_node_indexes(window)
    n1, n2 = ni[0], ni[1]
    weights_n = np.zeros_like(y_pred_np, dtype=np.float64)
    weights_p = np.zeros_like(y_pred_np, dtype=np.float64)
    half = window * (window - 1)
    for r in range(H_ // window):
        for c in range(W_ // window):
            rs = slice(r * window, (r + 1) * window)
            cs = slice(c * window, (c + 1) * window)
            Pw = y_pred_np[:, :, rs, cs]
            T = y_true_np[:, :, rs, cs]
            if Pw.min() == 1 or Pw.max() == 0:
                continue
            if T.min() == 1 or T.max() == 0:
                continue
            ch = (Pw[:, :, :, :-1] + Pw[:, :, :, 1:]).reshape(B_, -1)
            cv = (Pw[:, :, :-1, :] + Pw[:, :, 1:, :]).reshape(B_, -1)
            costs = np.concatenate([ch, cv], 1)
            gh = (T[:, :, :, :-1] + T[:, :, :, 1:]).reshape(B_, -1)
            gv = (T[:, :, :-1, :] + T[:, :, 1:, :]).reshape(B_, -1)
            gt = np.concatenate([gh, gv], 1)
            costs_n = costs.copy()
            costs_p = costs.copy()
            costs_n[gt > 20] = 20
            costs_p[gt < 10] = 0
            gt = np.minimum(gt, 20)
            for i in range(B_):
                seg = _label8(T[i, 0] == 0).astype(np.uint64)
                ewn = _malis_weights(seg, n1, n2, costs_n[i].astype(np.float32), 0).astype(np.float64)
                ewp = _malis_weights(seg, n1, n2, costs_p[i].astype(np.float32), 1).astype(np.float64)
                sn = ewn.sum()
                if sn > 0:
                    ewn = ewn / sn
                sp = ewp.sum()
                if sp > 0:
                    ewp = ewp / sp
                ewn[gt[i] >= 10] = 0
                ewp[gt[i] < 20] = 0
                for ew, Wacc in ((ewn, weights_n), (ewp, weights_p)):
                    wh = ew[:half].reshape(window, window - 1)
                    wv = ew[half:].reshape(window - 1, window)
                    blk = Wacc[i, 0, rs, cs]
                    blk[:, :-1] += wh
                    blk[:, 1:] += wh
                    blk[:-1, :] += wv
                    blk[1:, :] += wv
    return weights_n, weights_p


# ---------------------------------------------------------------------------
# Device kernel: per-core shard of sum(p^2*wn + (20-p)^2*wp)
# ---------------------------------------------------------------------------

_NC_CACHE = None

# test-harness knobs (the grading path leaves these untouched)
TRACE = False
LAST_RESULTS = None


def _build_bass():
    global _NC_CACHE
    if _NC_CACHE is not None:
        return _NC_CACHE

    nc = bass.Bass()
    f32 = mybir.dt.float32
    # one coalesced input: columns [p | wn | wp], each F wide
    x_d = nc.dram_tensor("x", [P, 3 * F], f32, kind="ExternalInput")
    out_d = nc.dram_tensor("out", [P, 1], f32, kind="ExternalOutput")

    with (
        nc.sbuf_tensor([P, 3 * F], f32) as tx,
        nc.sbuf_tensor([P, F], f32) as q1,
        nc.sbuf_tensor([P, F], f32) as sq,
        nc.sbuf_tensor([P, F], f32) as qq,
        nc.sbuf_tensor([P, 2 * F], f32) as buf,
        nc.sbuf_tensor([P, 1], f32) as r,
        nc.semaphore("d_sem") as d_sem,
        nc.semaphore("a_sem") as a_sem,
        nc.semaphore("v_sem") as v_sem,
        nc.Block() as block,
    ):
        tp = tx[:, 0:F]
        twn = tx[:, F:2 * F]
        twp = tx[:, 2 * F:3 * F]

        @block.sync
        def _(sync):
            sync.dma_start(tx[:], x_d[:]).then_inc(d_sem, 16)
            # wait rides the store itself; no trailing wait — the kernel-tail
            # drain flushes the HWDGE queue (then_inc required by codegen)
            sync.dma_start(out_d[:], r[:])._wait_ge(v_sem, 1).then_inc(d_sem, 16)

        @block.scalar
        def _(sc):
            sc.activation(
                sq[:], tp, mybir.ActivationFunctionType.Square
            )._wait_ge(d_sem, 16).then_inc(a_sem, 1)

        @block.vector
        def _(vector):
            # q1 = p - 20  (so q1^2 == (20-p)^2)
            vector.tensor_scalar_sub(q1[:], tp, 20.0)._wait_ge(d_sem, 16)
            vector.tensor_mul(qq[:], q1[:], q1[:])
            vector.tensor_mul(buf[:, F:], qq[:], twp)
            vector.tensor_mul(buf[:, :F], sq[:], twn)._wait_ge(a_sem, 1)
            vector.reduce_sum(r[:], buf[:], axis=mybir.AxisListType.X).then_inc(
                v_sem, 1
            )

    _NC_CACHE = nc
    return nc


def kernel(y_true: np.ndarray, y_pred: np.ndarray) -> np.ndarray:
    y_true = np.asarray(y_true, dtype=np.float32)
    y_pred = np.asarray(y_pred, dtype=np.float32)

    wn64, wp64 = _connectivity_weights(y_true, y_pred, WINDOW)
    wn = (MALIS_LR * wn64).astype(np.float32).reshape(-1)
    wp = (MALIS_LR_POS * wp64).astype(np.float32).reshape(-1)
    p = y_pred.reshape(-1)

    nc = _build_bass()
    in_maps = []
    for i in range(N_CORES):
        sl = slice(i * PER_CORE, (i + 1) * PER_CORE)
        x = np.concatenate(
            [p[sl].reshape(P, F), wn[sl].reshape(P, F), wp[sl].reshape(P, F)],
            axis=1,
        )
        in_maps.append({"x": np.ascontiguousarray(x)})

    res = run_bass_kernel_spmd(nc, in_maps, list(range(N_CORES)), trace=TRACE)
    global LAST_RESULTS
    LAST_RESULTS = res
    total = np.float64(0.0)
    for r in res.results:
        total += r["out"].astype(np.float64).sum()
    return np.asarray(total, dtype=np.float32)


# revision 13
# speedup vs baseline: 1.0086x; 1.0086x over previous
"""ConnectivityLoss (MALIS) kernel for Trainium2.

Contract mirrors the reference nn.Module: the MALIS pair weights are
computed on host (union-find / Kruskal maximin over prediction-dependent
edge costs, exactly as the torch module does via skimage+malis on detached
CPU numpy arrays); the device computes the weighted squared loss
    loss = sum( y_pred^2 * w_neg + (20 - y_pred)^2 * w_pos )
as a Bass kernel, data-parallel across 8 NeuronCores (each core reduces
a 16384-element shard; host adds the 8x[128,1] partials).
"""

import numpy as np

import concourse.bass as bass
import concourse.mybir as mybir
from concourse.bass_utils import run_bass_kernel_spmd

WINDOW = 32
MALIS_LR = 1.0
MALIS_LR_POS = 1.0

N_CORES = 8
B, C, H, W = 2, 1, 256, 256
TOTAL = B * C * H * W          # 131072
PER_CORE = TOTAL // N_CORES    # 16384
P = 128                        # SBUF partitions
F = PER_CORE // P              # 128 free elements per partition


# ---------------------------------------------------------------------------
# Host-side MALIS weights (exact replica of the reference host computation)
# ---------------------------------------------------------------------------

def _make_node_indexes(w):
    idx = np.arange(w * w).reshape(w, w)
    h = np.stack([idx[:, :-1].ravel(), idx[:, 1:].ravel()])
    v = np.stack([idx[:-1, :].ravel(), idx[1:, :].ravel()])
    return np.concatenate([h, v], axis=1)  # [2, 2*w*(w-1)]


class _UF:
    def __init__(self, n):
        self.p = list(range(n))

    def find(self, x):
        p = self.p
        while p[x] != x:
            p[x] = p[p[x]]
            x = p[x]
        return x


try:
    from scipy import ndimage as _ndimage
    _LABEL8_STRUCT = np.ones((3, 3), dtype=bool)
except Exception:  # pragma: no cover
    _ndimage = None


def _label8(mask):
    """8-connectivity labeling: labels>=1 inside mask, 0 outside.

    Only the label *partition* (which pixels share a component) matters for
    the MALIS pair counts — label numbering never affects the result.
    """
    if _ndimage is not None:
        labels, _ = _ndimage.label(mask, structure=_LABEL8_STRUCT)
        return labels.astype(np.int64).ravel()
    h, w = mask.shape
    n = h * w
    uf = _UF(n)
    idx = np.arange(n).reshape(h, w)
    pairs = []
    for (a, b) in (
        (idx[:, :-1], idx[:, 1:]),
        (idx[:-1, :], idx[1:, :]),
        (idx[:-1, :-1], idx[1:, 1:]),
        (idx[:-1, 1:], idx[1:, :-1]),
    ):
        m = mask.ravel()[a.ravel()] & mask.ravel()[b.ravel()]
        pairs.append(np.stack([a.ravel()[m], b.ravel()[m]], 1))
    for a, b in np.concatenate(pairs, 0):
        ra, rb = uf.find(int(a)), uf.find(int(b))
        if ra != rb:
            uf.p[rb] = ra
    labels = np.zeros(n, dtype=np.int64)
    root_lab = {}
    nxt = 1
    flat = mask.ravel()
    for i in range(n):
        if flat[i]:
            r = uf.find(i)
            if r not in root_lab:
                root_lab[r] = nxt
                nxt += 1
            labels[i] = root_lab[r]
    return labels


def _malis_weights(seg, n1, n2, costs, pos):
    """MALIS pair counts per edge: Kruskal maximin, descending cost order.

    Exact-equivalent rewrite of the straightforward version: pair counts are
    symmetric in the two merged components (pos: sum_l |A_l|*|B_l|; neg:
    |A|*|B| - sum_l |A_l|*|B_l|), so merge direction and union-find internals
    are free — only the stable descending edge order matters.
    """
    n = seg.shape[0]
    parent = list(range(n))
    seg_l = seg.astype(np.int64).tolist()
    overlap = [({s: 1} if s else None) for s in seg_l]
    tot = [1 if s else 0 for s in seg_l]
    w = np.zeros(costs.shape[0], dtype=np.int64)
    order = np.argsort(-costs, kind="stable").tolist()
    n1l = n1.tolist()
    n2l = n2.tolist()
    for e in order:
        a = n1l[e]
        b = n2l[e]
        while parent[a] != a:
            parent[a] = parent[parent[a]]
            a = parent[a]
        while parent[b] != b:
            parent[b] = parent[parent[b]]
            b = parent[b]
        if a == b:
            continue
        oa = overlap[a]
        ob = overlap[b]
        if oa is None:
            parent[a] = b
            continue
        if ob is None:
            parent[b] = a
            continue
        ta = tot[a]
        tb = tot[b]
        if len(oa) >= len(ob):
            big, small = oa, ob
        else:
            big, small = ob, oa
        inner = 0
        for l, c in small.items():
            g = big.get(l)
            if g:
                inner += c * g
        w[e] = inner if pos else ta * tb - inner
        for l, c in small.items():
            big[l] = big.get(l, 0) + c
        if big is oa:
            parent[b] = a
            tot[a] = ta + tb
            overlap[b] = None
        else:
            parent[a] = b
            tot[b] = ta + tb
            overlap[a] = None
    return w


def _connectivity_weights(y_true_np, y_pred_np, window):
    B_, C_, H_, W_ = y_pred_np.shape
    ni = _make_node_indexes(window)
    n1, n2 = ni[0], ni[1]
    weights_n = np.zeros_like(y_pred_np, dtype=np.float64)
    weights_p = np.zeros_like(y_pred_np, dtype=np.float64)
    half = window * (window - 1)
    for r in range(H_ // window):
        for c in range(W_ // window):
            rs = slice(r * window, (r + 1) * window)
            cs = slice(c * window, (c + 1) * window)
            Pw = y_pred_np[:, :, rs, cs]
            T = y_true_np[:, :, rs, cs]
            if Pw.min() == 1 or Pw.max() == 0:
                continue
            if T.min() == 1 or T.max() == 0:
                continue
            ch = (Pw[:, :, :, :-1] + Pw[:, :, :, 1:]).reshape(B_, -1)
            cv = (Pw[:, :, :-1, :] + Pw[:, :, 1:, :]).reshape(B_, -1)
            costs = np.concatenate([ch, cv], 1)
            gh = (T[:, :, :, :-1] + T[:, :, :, 1:]).reshape(B_, -1)
            gv = (T[:, :, :-1, :] + T[:, :, 1:, :]).reshape(B_, -1)
            gt = np.concatenate([gh, gv], 1)
            costs_n = costs.copy()
            costs_p = costs.copy()
            costs_n[gt > 20] = 20
            costs_p[gt < 10] = 0
            gt = np.minimum(gt, 20)
            for i in range(B_):
                seg = _label8(T[i, 0] == 0).astype(np.uint64)
                ewn = _malis_weights(seg, n1, n2, costs_n[i].astype(np.float32), 0).astype(np.float64)
                ewp = _malis_weights(seg, n1, n2, costs_p[i].astype(np.float32), 1).astype(np.float64)
                sn = ewn.sum()
                if sn > 0:
                    ewn = ewn / sn
                sp = ewp.sum()
                if sp > 0:
                    ewp = ewp / sp
                ewn[gt[i] >= 10] = 0
                ewp[gt[i] < 20] = 0
                for ew, Wacc in ((ewn, weights_n), (ewp, weights_p)):
                    wh = ew[:half].reshape(window, window - 1)
                    wv = ew[half:].reshape(window - 1, window)
                    blk = Wacc[i, 0, rs, cs]
                    blk[:, :-1] += wh
                    blk[:, 1:] += wh
                    blk[:-1, :] += wv
                    blk[1:, :] += wv
    return weights_n, weights_p


# ---------------------------------------------------------------------------
# Device kernel: per-core shard of sum(p^2*wn + (20-p)^2*wp)
# ---------------------------------------------------------------------------

_NC_CACHE = None

# test-harness knobs (the grading path leaves these untouched)
TRACE = False
LAST_RESULTS = None


def _build_bass():
    global _NC_CACHE
    if _NC_CACHE is not None:
        return _NC_CACHE

    nc = bass.Bass(monotonic_sem_count=0)
    f32 = mybir.dt.float32
    # one coalesced input: columns [p | wn | wp], each F wide
    x_d = nc.dram_tensor("x", [P, 3 * F], f32, kind="ExternalInput")
    out_d = nc.dram_tensor("out", [P, 1], f32, kind="ExternalOutput")

    with (
        nc.sbuf_tensor([P, 3 * F], f32) as tx,
        nc.sbuf_tensor([P, F], f32) as q1,
        nc.sbuf_tensor([P, F], f32) as sq,
        nc.sbuf_tensor([P, F], f32) as qq,
        nc.sbuf_tensor([P, 2 * F], f32) as buf,
        nc.sbuf_tensor([P, 1], f32) as r,
        nc.semaphore("d_sem") as d_sem,
        nc.semaphore("a_sem") as a_sem,
        nc.semaphore("v_sem") as v_sem,
        nc.Block() as block,
    ):
        tp = tx[:, 0:F]
        twn = tx[:, F:2 * F]
        twp = tx[:, 2 * F:3 * F]

        @block.sync
        def _(sync):
            sync.dma_start(tx[:], x_d[:]).then_inc(d_sem, 16)
            # wait rides the store itself; no trailing wait — the kernel-tail
            # drain flushes the HWDGE queue (then_inc required by codegen)
            sync.dma_start(out_d[:], r[:])._wait_ge(v_sem, 1).then_inc(d_sem, 16)

        @block.scalar
        def _(sc):
            sc.activation(
                sq[:], tp, mybir.ActivationFunctionType.Square
            )._wait_ge(d_sem, 16).then_inc(a_sem, 1)

        @block.vector
        def _(vector):
            # q1 = p - 20  (so q1^2 == (20-p)^2)
            vector.tensor_scalar_sub(q1[:], tp, 20.0)._wait_ge(d_sem, 16)
            vector.tensor_mul(qq[:], q1[:], q1[:])
            vector.tensor_mul(buf[:, F:], qq[:], twp)
            vector.tensor_mul(buf[:, :F], sq[:], twn)._wait_ge(a_sem, 1)
            vector.reduce_sum(r[:], buf[:], axis=mybir.AxisListType.X).then_inc(
                v_sem, 1
            )

    _NC_CACHE = nc
    return nc


def kernel(y_true: np.ndarray, y_pred: np.ndarray) -> np.ndarray:
    y_true = np.asarray(y_true, dtype=np.float32)
    y_pred = np.asarray(y_pred, dtype=np.float32)

    wn64, wp64 = _connectivity_weights(y_true, y_pred, WINDOW)
    wn = (MALIS_LR * wn64).astype(np.float32).reshape(-1)
    wp = (MALIS_LR_POS * wp64).astype(np.float32).reshape(-1)
    p = y_pred.reshape(-1)

    nc = _build_bass()
    in_maps = []
    for i in range(N_CORES):
        sl = slice(i * PER_CORE, (i + 1) * PER_CORE)
        x = np.concatenate(
            [p[sl].reshape(P, F), wn[sl].reshape(P, F), wp[sl].reshape(P, F)],
            axis=1,
        )
        in_maps.append({"x": np.ascontiguousarray(x)})

    res = run_bass_kernel_spmd(nc, in_maps, list(range(N_CORES)), trace=TRACE)
    global LAST_RESULTS
    LAST_RESULTS = res
    total = np.float64(0.0)
    for r in res.results:
        total += r["out"].astype(np.float64).sum()
    return np.asarray(total, dtype=np.float32)
